# revision 11
# baseline (speedup 1.0000x reference)
"""GNN message-passing kernel for Trainium2 (8 NeuronCores, SPMD).

Computes: out = segment_sum(x[edge_index[0]], edge_index[1], num_segments=N)
  i.e. for each edge e: out[dst[e]] += x[src[e]]

Strategy:
  - Shard destination nodes across 8 cores (R=12800 nodes/core, padded space).
  - x replicated; each core gathers its edges' source rows from HBM via the
    custom Q7 dma_gather (int16 idx, 4 chunk bases of 32768 rows, 4 SWDGE
    queues for parallel descriptor service).
  - Edges sorted by (dst bigwin, src chunk, dst). One gather call per
    (bigwin=512 dst nodes, chunk). Tiles of 128 edge slots aligned to
    subwindows (128 dst nodes) with -1 slot padding (free: skipped by DMA).
  - Scatter: per tile, DVE builds a one-hot [128 edges, 128 nodes] via
    is_equal(dst_local, iota); PE matmul accumulates msgs^T @ onehot into a
    feature-major PSUM bank [64 feat, 512 nodes] per bigwin.
  - Flush PSUM -> SBUF outT [64, 12800] -> single DMA out; host transposes.

The Bass program is identical across cores (SPMD): tile counts use the max
over cores; per-core valid-index counts are runtime inputs consumed via
reg_load into the dma_gather num_idxs_reg.
"""
import numpy as np

N_NODES = 100000
D_FEAT = 64
N_CORES = 8
R = 12800            # dst nodes per core (8*R = 102400 >= N_NODES)
BW = 512             # bigwin: dst nodes per PSUM bank
SW = 128             # subwindow: dst nodes per tile target / onehot width
CHUNK = 32768        # src rows per gather base (int16 idx limit)
N_CHUNKS = 4         # ceil(100000 / 32768)
N_BW = R // BW       # 25 bigwins per core
N_SW = BW // SW      # 4 subwindows per bigwin
P = 128              # edge slots per tile


def cdiv(a, b):
    return -(-a // b)


def _preprocess(x, edge_index):
    """Sort/partition edges; build per-core device input streams and the
    (core-uniform) schedule."""
    src = np.asarray(edge_index[0], dtype=np.int64)
    dst = np.asarray(edge_index[1], dtype=np.int64)
    E = src.shape[0]

    core = dst // R
    dloc = dst % R                 # dst local to core
    bw = dloc // BW                # bigwin within core
    sw = (dloc % BW) // SW         # subwindow within bigwin
    ck = src // CHUNK              # src chunk

    # group id per edge: (core, bw, ck, sw)
    gid = ((core * N_BW + bw) * N_CHUNKS + ck) * N_SW + sw
    n_groups = N_CORES * N_BW * N_CHUNKS * N_SW
    order = np.argsort(gid, kind="stable")
    gid_s = gid[order]
    src_s = src[order]
    dloc_s = dloc[order]

    counts = np.bincount(gid_s, minlength=n_groups)           # [n_groups]
    counts4 = counts.reshape(N_CORES, N_BW, N_CHUNKS, N_SW)
    # tiles per (bw, ck, sw): max over cores (uniform program)
    T_g = cdiv(counts4.max(axis=0), P)                        # [N_BW, N_CHUNKS, N_SW]

    # schedule: one gather call per nonempty group (bw, ck, sw);
    # padding is a -1 suffix per call (trimmed by the Q7 ucode, no DMA cost).
    tiles_per_call = T_g.sum(axis=2)                          # [N_BW, N_CHUNKS]
    n_calls = N_BW * N_CHUNKS
    tot_tiles = int(T_g.sum())
    tot_slots = tot_tiles * P

    # slot offset of each group (bw, ck, sw) within the global stream
    grp_tile_off = np.zeros((N_BW, N_CHUNKS, N_SW), np.int64)
    acc = 0
    for b in range(N_BW):
        for c in range(N_CHUNKS):
            for s in range(N_SW):
                grp_tile_off[b, c, s] = acc
                acc += T_g[b, c, s]
    assert acc == tot_tiles
    grp_slot_off = grp_tile_off * P

    # tile metadata (uniform across cores): subwindow index per global tile
    tile_sw = np.zeros(tot_tiles, np.int64)
    tile_bw = np.zeros(tot_tiles, np.int64)
    tile_call = np.zeros(tot_tiles, np.int64)
    for b in range(N_BW):
        for c in range(N_CHUNKS):
            for s in range(N_SW):
                o = grp_tile_off[b, c, s]
                t = T_g[b, c, s]
                tile_sw[o:o + t] = s
                tile_bw[o:o + t] = b
                tile_call[o:o + t] = b * N_CHUNKS + c

    # per-edge slot assignment (vectorized)
    # rank of edge within its group:
    grp_start_edge = np.zeros(n_groups + 1, np.int64)
    np.cumsum(counts, out=grp_start_edge[1:])
    rank = np.arange(E, dtype=np.int64) - grp_start_edge[gid_s]
    b_e = (gid_s // (N_CHUNKS * N_SW)) % N_BW
    c_e = (gid_s // N_SW) % N_CHUNKS
    s_e = gid_s % N_SW
    slot = grp_slot_off[b_e, c_e, s_e] + rank                  # within-core slot
    core_e = gid_s // (N_BW * N_CHUNKS * N_SW)

    # build per-core streams
    idx16_cores, dstl_cores, cnt_cores = [], [], []
    for cr in range(N_CORES):
        m = core_e == cr
        sl = slot[m]
        padvalid = bool(__import__('os').environ.get('PADVALID'))
        stream = np.full(tot_slots, 0 if padvalid else -1, np.int16)
        lidx = (src_s[m] - c_e[m] * CHUNK).astype(np.int16)
        stream[sl] = lidx
        # wrapped-16 idx layout, replicated across 8 groups of 16 partitions
        wrapped = stream.reshape(tot_slots // 16, 16).T        # [16, S/16]
        idx16 = np.tile(wrapped, (8, 1)).astype(np.int16)      # [128, S/16]
        idx16_cores.append(idx16)

        dstl = np.full(tot_slots, -1.0, np.float32)
        dstl[sl] = (dloc_s[m] % SW).astype(np.float32)
        dstl = dstl.reshape(tot_tiles, P).T.copy()             # [128, tot_tiles]
        dstl_cores.append(dstl)

        # valid count per group, flattened (b, c, s)
        cnt = counts4[cr].reshape(-1).astype(np.int32)
        cnt_cores.append(cnt.reshape(1, -1))

    sched = dict(
        T_g=T_g, tiles_per_call=tiles_per_call,
        grp_tile_off=grp_tile_off,
        tot_slots=tot_slots, tot_tiles=tot_tiles, n_calls=n_calls,
        tile_sw=tile_sw, tile_bw=tile_bw, tile_call=tile_call,
    )
    return sched, idx16_cores, dstl_cores, cnt_cores


def _build_program(sched, n_x_rows):
    import concourse.bass as bass
    import concourse.bacc as bacc
    import concourse.mybir as mybir
    import concourse.tile as tile

    tot_slots = sched["tot_slots"]
    tot_tiles = sched["tot_tiles"]
    T_g = sched["T_g"]
    grp_tile_off = sched["grp_tile_off"]

    max_grp_tiles = int(T_g.max())

    nc = bacc.Bacc(None, target_bir_lowering=False, debug=False,
                   num_swdge_queues=4)
    x_in = nc.declare_dram_parameter("x", [n_x_rows, D_FEAT], mybir.dt.float32,
                                     isOutput=False)
    idx_in = nc.declare_dram_parameter("idx", [128, tot_slots // 16],
                                       mybir.dt.int16, isOutput=False)
    dstl_in = nc.declare_dram_parameter("dstl", [128, tot_tiles],
                                        mybir.dt.float32, isOutput=False)
    iota_in = nc.declare_dram_parameter("iota", [128, SW], mybir.dt.float32,
                                        isOutput=False)
    n_groups = N_BW * N_CHUNKS * N_SW
    cnt_in = nc.declare_dram_parameter("cnt", [1, n_groups], mybir.dt.int32,
                                       isOutput=False)
    yT_out = nc.declare_dram_parameter("yT", [D_FEAT, R], mybir.dt.float32,
                                       isOutput=True)

    with tile.TileContext(nc) as tc:
        with (
            tc.tile_pool(name="const", bufs=1) as constp,
            tc.tile_pool(name="idxp", bufs=1) as idxp,
            tc.tile_pool(name="dstlp", bufs=1) as dstlp,
            tc.tile_pool(name="outp", bufs=1) as outp,
            tc.tile_pool(name="msgp", bufs=1) as msgp,
            tc.tile_pool(name="ohp", bufs=8) as ohp,
            tc.tile_pool(name="psp", bufs=4, space="PSUM") as psp,
        ):
            iota_sb = constp.tile([128, SW], mybir.dt.float32)
            nc.sync.dma_start(out=iota_sb[:], in_=iota_in[:, :])
            zero64 = constp.tile([128, D_FEAT], mybir.dt.float32)
            nc.gpsimd.memset(zero64[:], 0.0)
            zrhs = constp.tile([128, BW], mybir.dt.float32)
            nc.gpsimd.memset(zrhs[:], 0.0)
            idx_sb = idxp.tile([128, tot_slots // 16], mybir.dt.int16)
            nc.sync.dma_start(out=idx_sb[:], in_=idx_in[:, :])
            dstl_sb = dstlp.tile([128, tot_tiles], mybir.dt.float32)
            nc.sync.dma_start(out=dstl_sb[:], in_=dstl_in[:, :])
            outT_sb = outp.tile([D_FEAT, R], mybir.dt.float32)
            cnt_sb = constp.tile([1, n_groups], mybir.dt.int32)
            nc.sync.dma_start(out=cnt_sb[:], in_=cnt_in[:, :])
            reg = nc.gpsimd.alloc_register("nval")
            prev_gather = None

            # manual msg ring: memset once so -1-padded slots stay finite
            N_MSG_BUFS = 12
            msg_bufs = []
            for i in range(N_MSG_BUFS):
                mb = msgp.tile([128, max_grp_tiles, D_FEAT], mybir.dt.float32,
                               tag=f"msg{i}")
                nc.vector.memset(mb[:], 0.0)
                msg_bufs.append(mb)

            qn = 0
            for b in range(N_BW):
                psumT_full = psp.tile([128, BW], mybir.dt.float32, space="PSUM")
                psumT = psumT_full[0:D_FEAT, :]
                # zero-fill the bank (handles zero-edge node columns)
                nc.tensor.matmul(out=psumT, lhsT=zero64[:], rhs=zrhs[:],
                                 start=True, stop=False)
                bw_tiles = int(sched["tiles_per_call"][b, :].sum())
                done = 0
                for c in range(N_CHUNKS):
                    for sw_i in range(N_SW):
                        T = int(T_g[b, c, sw_i])
                        if T == 0:
                            continue
                        gt0 = int(grp_tile_off[b, c, sw_i])
                        s0 = gt0 * P
                        S = T * P
                        msg = msg_bufs[qn % N_MSG_BUFS]
                        gidx = (b * N_CHUNKS + c) * N_SW + sw_i
                        ld = nc.gpsimd.reg_load(reg, cnt_sb[0:1, gidx:gidx + 1])
                        g = nc.gpsimd.dma_gather(
                            out_ap=msg[:, :T, :],
                            in_ap=x_in[c * CHUNK:, :],
                            idxs_ap=idx_sb[:, s0 // 16:(s0 + S) // 16],
                            num_idxs=S,
                            num_idxs_reg=reg,
                            elem_size=D_FEAT,
                            single_packet=False,
                            queue_num=qn % 4,
                        )
                        tile.add_dep_helper(g.ins, ld.ins, sync=False,
                                            reason="gather reads nval reg")
                        if prev_gather is not None:
                            tile.add_dep_helper(ld.ins, prev_gather.ins,
                                                sync=False,
                                                reason="reg reuse ordering")
                        prev_gather = g
                        qn += 1
                        for tl in range(T):
                            gt = gt0 + tl
                            oh = ohp.tile([128, SW], mybir.dt.float32, tag="oh")
                            nc.vector.tensor_tensor(
                                out=oh[:],
                                in0=dstl_sb[:, gt:gt + 1].to_broadcast([128, SW]),
                                in1=iota_sb[:],
                                op=mybir.AluOpType.is_equal,
                            )
                            done += 1
                            nc.tensor.matmul(
                                out=psumT[0:D_FEAT, sw_i * SW:(sw_i + 1) * SW],
                                lhsT=msg[:, tl, :],
                                rhs=oh[:],
                                start=False,
                                stop=(done == bw_tiles),
                            )
                nc.vector.tensor_copy(out=outT_sb[:, b * BW:(b + 1) * BW],
                                      in_=psumT)
            nc.sync.dma_start(out=yT_out[:, :], in_=outT_sb[:])
    nc.compile()
    return nc


def build(x, edge_index):
    """Preprocess + build the compiled Bass program and per-core input maps."""
    x = np.ascontiguousarray(np.asarray(x, dtype=np.float32))
    edge_index = np.asarray(edge_index)
    assert x.shape[1] == D_FEAT, x.shape

    sched, idx16_cores, dstl_cores, cnt_cores = _preprocess(x, edge_index)
    nc = _build_program(sched, x.shape[0])

    iota = np.tile(np.arange(SW, dtype=np.float32), (128, 1))
    in_maps = []
    for cr in range(N_CORES):
        in_maps.append({
            "x": x,
            "idx": idx16_cores[cr],
            "dstl": dstl_cores[cr],
            "cnt": cnt_cores[cr],
            "iota": iota,
        })
    return nc, in_maps


def postprocess(results, n_nodes):
    out = np.empty((N_CORES * R, D_FEAT), np.float32)
    for cr in range(N_CORES):
        out[cr * R:(cr + 1) * R] = results[cr]["yT"].T
    return out[:n_nodes]


def kernel(x, edge_index):
    n_nodes = np.asarray(x).shape[0]
    nc, in_maps = build(x, edge_index)
    from concourse.bass_utils import run_bass_kernel_spmd
    res = run_bass_kernel_spmd(nc, in_maps, list(range(N_CORES)))
    return postprocess(res.results, n_nodes)


if __name__ == "__main__":
    import reference
    inputs = reference.setup_inputs()
    inputs = {k: np.asarray(v) for k, v in inputs.items()}
    got = kernel(**inputs)
    want = np.asarray(reference.reference(**{k: v for k, v in inputs.items()}))
    denom = max(np.abs(want).max(), 1e-30)
    rel = np.abs(got - want).max() / denom
    print(f"Relative error: {rel:.3e}")
